# revision 8
# baseline (speedup 1.0000x reference)
"""Trainium2 Bass kernel for nn_MultiHeadAttention_60258391163205.

Causal multi-head attention (B=2, S=2048, E=1024, H=16 heads, D=64),
fp32 inputs/outputs.

Sharding (Megatron-style, per the hint): 8 cores = data-parallel over the
2 batches x tensor-parallel over 4 head-groups (4 heads each).  Each core
gets Wq/Wk/Wv column-shards and the matching Wo row-shard, computes its
heads' attention for its batch, and emits a PARTIAL output projection
(2048, 1024).  The host sums the 4 partials per batch and adds bo.

Device algorithm (per core), all matmuls in float32r (TF32-like fast fp32):
  - host pre-transposes x[b] -> xT (E on partitions) so every contraction
    has its reduction dim on SBUF partitions.
  - qT/kT = (x @ Wq/k + b).T computed directly in [e', s] layout;
    v computed in natural [s, e'] layout interleaved per head with a ones
    column appended (v_ext[., 64] = 1).
  - scores are computed TRANSPOSED, eT[j, i] = exp((k_j . q_i)/32), so
    softmax never needs a partition reduction and p is never transposed:
      u[d, i] (+ l[i] via the ones column) = v_ext.T @ eT  accumulated in
    PSUM over j-tiles; the causal mask is applied as (a) skipping fully
    masked (j, i) blocks, (b) narrowing partial blocks, (c) one 128x128
    triangular elementwise mask on the diagonal block.
  - attnT[hd, i] = u * (1/l) with 1/l broadcast across partitions via a
    tiny SBUF->SBUF DMA; out_partial[i, f] = attnT.T @ Wo_shard.

Numerics: float32r rounds operands to ~12 mantissa bits before the MAC
(measured max rel err ~8e-4 on a K=1024 matmul); softmax skips the
max-subtraction since |scores/32| < ~2 for these inputs, which is exact in
fp32 terms.  End-to-end rel err vs the fp32 reference is ~1e-3.

This walrus build accepts only ONE semaphore wait per instruction
("Too many sync wait commands"); _split_multi_waits() hoists extra waits
emitted by Tile onto same-engine NoOps, which is semantically identical
because engine streams execute in order.
"""

import sys

if "/opt/trn_rl_repo" not in sys.path:
    sys.path.insert(0, "/opt/trn_rl_repo")

import numpy as np

import bass_rust
import concourse.bass as bass
import concourse.mybir as mybir
import concourse.tile as tile

B, S, E, H, D = 2, 2048, 1024, 16, 64
NCORES = 8
TP = 4                      # head-group shards
HG = H // TP                # heads per core = 4
EG = HG * D                 # e' columns per core = 256
F32 = mybir.dt.float32
F32R = mybir.dt.float32r
FP = mybir.dt  # short alias
AX = mybir.AluOpType
ACTF = mybir.ActivationFunctionType

SCALE = 1.0 / np.sqrt(np.float32(E)).astype(np.float32)  # 1/32 exact

KT = E // 128               # 8 contraction k-tiles
ST = S // 128               # 16 s-tiles of 128
SC = S // 512               # 4 s-chunks of 512
EXPG = 3                    # score blocks exp'd per ACT call (3 psum banks)


def _split_multi_waits(nc):
    """Walrus here accepts a single sem-wait per instruction; hoist extras
    onto same-engine NoOps placed immediately before (streams are in-order,
    so semantics are unchanged)."""
    n = 0
    for fn in nc.m.functions:
        for bb in fn.blocks:
            out = []
            for ins in bb.instructions:
                si = ins.sync_info
                if si is not None and si.on_wait and len(si.on_wait) > 1:
                    waits = list(si.on_wait)
                    for w in waits[:-1]:
                        nop = bass_rust.InstNoOp(name=f"I-waitfix-{nc.next_id()}")
                        nop.engine = ins.engine
                        nop.sync_info = mybir.SyncInfo(on_wait=[w], on_update=[])
                        out.append(nop)
                    si.on_wait = waits[-1:]
                    n += 1
                out.append(ins)
            bb.instructions = out
    return n


def build_nc():
    nc = bass.Bass()

    xT = nc.dram_tensor("xT", [E, S], F32R, kind="ExternalInput")
    wq = nc.dram_tensor("wq", [E, EG], F32R, kind="ExternalInput")
    wk = nc.dram_tensor("wk", [E, EG], F32R, kind="ExternalInput")
    wv = nc.dram_tensor("wv", [E, EG], F32R, kind="ExternalInput")
    wo = nc.dram_tensor("wo", [EG, E], F32R, kind="ExternalInput")
    bqd = nc.dram_tensor("bq", [EG], F32, kind="ExternalInput")
    bkd = nc.dram_tensor("bk", [EG], F32, kind="ExternalInput")
    bvd = nc.dram_tensor("bv", [EG], F32, kind="ExternalInput")
    trid = nc.dram_tensor("trimask", [128, 128], F32R, kind="ExternalInput")
    out = nc.dram_tensor("out", [S, E], F32, kind="ExternalOutput")

    x3 = xT.rearrange("(ko ki) s -> ki ko s", ki=128)
    wq3 = wq.rearrange("(ko ki) m -> ki ko m", ki=128)
    wk3 = wk.rearrange("(ko ki) m -> ki ko m", ki=128)
    wv3 = wv.rearrange("(ko ki) m -> ki ko m", ki=128)
    wo3 = wo.rearrange("(to ti) f -> ti to f", ti=128)

    with tile.TileContext(nc) as tc:
        with (
            tc.tile_pool(name="consts", bufs=1) as consts,
            tc.tile_pool(name="acts", bufs=1) as acts,
            tc.tile_pool(name="ep", bufs=2) as ep,
            tc.tile_pool(name="rp", bufs=3) as rp,
            tc.tile_pool(name="stg", bufs=3) as stg,
            tc.tile_pool(name="dr", bufs=2, space="DRAM") as dr,
            tc.tile_pool(name="psA", bufs=2, space="PSUM") as psA,
            tc.tile_pool(name="psU", bufs=2, space="PSUM") as psU,
        ):
            # ---- constants / weights in SBUF ----
            w_sb = {}
            for nm, dram in (("wq", wq3), ("wk", wk3), ("wv", wv3)):
                t = consts.tile([128, KT, EG], F32R, tag=nm)
                nc.sync.dma_start(t[:], dram[:])
                w_sb[nm] = t
            wo_sb = consts.tile([128, 2, E], F32R, tag="wo")
            nc.sync.dma_start(wo_sb[:], wo3[:])
            bq_sb = consts.tile([128, 2], F32, tag="bq")
            nc.sync.dma_start(bq_sb[:], bqd.rearrange("(t p) -> p t", p=128))
            bk_sb = consts.tile([128, 2], F32, tag="bk")
            nc.sync.dma_start(bk_sb[:], bkd.rearrange("(t p) -> p t", p=128))
            bv_sb = consts.tile([128, EG], F32, tag="bv")
            nc.sync.dma_start(bv_sb[:], bvd[None, :].partition_broadcast(128))
            tri_sb = consts.tile([128, 128], F32R, tag="tri")
            nc.sync.dma_start(tri_sb[:], trid[:])

            x_sb = acts.tile([128, KT, S], F32R, tag="xT")
            for kt in range(KT):
                nc.sync.dma_start(x_sb[:, kt, :], x3[:, kt, :])

            qT = acts.tile([128, 2, S], F32R, tag="qT")
            kTt = acts.tile([128, 2, S], F32R, tag="kT")
            # v interleaved per head with a ones column: [s%128, s//128, h, 65]
            v_sb = acts.tile([128, ST, HG, D + 1], F32R, tag="v")
            attnT = acts.tile([128, 2, S], F32R, tag="attnT")

            # (memset can't target f32r -- write the 1.0s through an f32 view)
            nc.vector.memset(v_sb[:, :, :, D : D + 1].bitcast(F32), 1.0)

            # ---- V projection (natural layout) ----
            with nc.named_scope("vproj"):
                for st_i in range(ST):
                    pv = psA.tile([128, EXPG, 512], F32, tag="psA")
                    for kt in range(KT):
                        nc.tensor.matmul(
                            pv[:, 0, 0:EG],
                            x_sb[:, kt, st_i * 128 : (st_i + 1) * 128],
                            w_sb["wv"][:, kt, :],
                            start=(kt == 0),
                            stop=(kt == KT - 1),
                        )
                    nc.vector.tensor_tensor(
                        out=v_sb[:, st_i, :, 0:D],
                        in0=pv[:, 0, 0:EG].rearrange("p (h d) -> p h d", h=HG),
                        in1=bv_sb[:].rearrange("p (h d) -> p h d", h=HG),
                        op=AX.add,
                    )

            # ---- Q/K projections (transposed layout) ----
            with nc.named_scope("qkproj"):
                for t in range(2):
                    for nm, dst, b_sb in (("wq", qT, bq_sb), ("wk", kTt, bk_sb)):
                        for cg in range(2):  # two [128, 2, 512] psum groups
                            p = psA.tile([128, EXPG, 512], F32, tag="psA")
                            for sc in range(2):
                                schunk = cg * 2 + sc
                                for kt in range(KT):
                                    nc.tensor.matmul(
                                        p[:, sc, :],
                                        w_sb[nm][:, kt, t * 128 : (t + 1) * 128],
                                        x_sb[:, kt, schunk * 512 : (schunk + 1) * 512],
                                        start=(kt == 0),
                                        stop=(kt == KT - 1),
                                    )
                            nc.vector.tensor_scalar(
                                out=dst[:, t, cg * 1024 : (cg + 1) * 1024],
                                in0=p[:, 0:2, :].rearrange("p a n -> p (a n)"),
                                scalar1=b_sb[:, t : t + 1],
                                scalar2=None,
                                op0=AX.add,
                            )

            # ---- attention, scores transposed, flash-style over j ----
            with nc.named_scope("attn"):
                for t in range(2):
                    for hl in range(2):
                        hh = t * 2 + hl
                        r0, r1 = hl * D, (hl + 1) * D
                        for it4 in range(SC):
                            i0 = it4 * 512
                            pu = psU.tile([65, 512], F32, tag="psU")
                            jts = list(range(4 * it4 + 4))
                            groups = [
                                jts[a : a + EXPG] for a in range(0, len(jts), EXPG)
                            ]
                            for grp in groups:
                                ps = psA.tile([128, EXPG, 512], F32, tag="psA")
                                et = ep.tile([128, EXPG, 512], F32R, tag="eT")
                                for q, jt in enumerate(grp):
                                    m = jt - 4 * it4
                                    off = 128 * m if m > 0 else 0
                                    nc.tensor.matmul(
                                        ps[:, q, off:512],
                                        kTt[r0:r1, t, jt * 128 : (jt + 1) * 128],
                                        qT[r0:r1, t, i0 + off : i0 + 512],
                                        start=True,
                                        stop=True,
                                    )
                                ng = len(grp)
                                nc.scalar.activation(
                                    out=et[:, 0:ng, :],
                                    in_=ps[:, 0:ng, :],
                                    func=ACTF.Exp,
                                    scale=float(SCALE),
                                )
                                for q, jt in enumerate(grp):
                                    m = jt - 4 * it4
                                    if m >= 0:
                                        off = 128 * m
                                        nc.vector.tensor_tensor(
                                            out=et[:, q, off : off + 128],
                                            in0=et[:, q, off : off + 128],
                                            in1=tri_sb[:],
                                            op=AX.mult,
                                        )
                                for q, jt in enumerate(grp):
                                    m = jt - 4 * it4
                                    off = 128 * m if m > 0 else 0
                                    nc.tensor.matmul(
                                        pu[:, off:512],
                                        v_sb[:, jt, hh, :],
                                        et[:, q, off:512],
                                        start=(jt == 0),
                                        stop=(jt == jts[-1]),
                                    )
                            # normalize: attnT[hd, i] = u[d, i] / l[i]
                            # (1/l broadcast across partitions via a DRAM
                            # bounce -- SBUF APs can't have partition step 0)
                            rrow = rp.tile([1, 512], F32, tag="rrow")
                            nc.vector.reciprocal(rrow[:], pu[64:65, :])
                            rdr = dr.tile([1, 512], F32, tag="rdr")
                            nc.sync.dma_start(rdr[:], rrow[:])
                            rb = rp.tile([64, 512], F32, tag="rb")
                            nc.sync.dma_start(
                                rb[:],
                                bass.AP(
                                    tensor=rdr.tensor,
                                    offset=rdr.offset,
                                    ap=[[0, 64]] + list(rdr.ap[1:]),
                                ),
                            )
                            nc.vector.tensor_tensor(
                                out=attnT[r0:r1, t, i0 : i0 + 512],
                                in0=pu[0:D, :],
                                in1=rb[:],
                                op=AX.mult,
                            )

            # ---- output projection (partial) ----
            with nc.named_scope("oproj"):
                out3 = out.rearrange("(io p) f -> p io f", p=128)
                for it in range(ST):
                    po = psA.tile([128, EXPG, 512], F32, tag="psA")
                    for fc in range(2):
                        for t in range(2):
                            nc.tensor.matmul(
                                po[:, fc, :],
                                attnT[:, t, it * 128 : (it + 1) * 128],
                                wo_sb[:, t, fc * 512 : (fc + 1) * 512],
                                start=(t == 0),
                                stop=(t == 1),
                            )
                    so = stg.tile([128, E], F32, tag="so")
                    nc.vector.tensor_copy(
                        so[:], po[:, 0:2, :].rearrange("p a n -> p (a n)")
                    )
                    nc.sync.dma_start(out3[:, it, :], so[:])

    _split_multi_waits(nc)
    return nc


_NC_CACHE = None


def _get_nc():
    global _NC_CACHE
    if _NC_CACHE is None:
        _NC_CACHE = build_nc()
    return _NC_CACHE


def make_in_maps(x, Wq, bq, Wk, bk, Wv, bv, Wo, bo):
    # scores are stored transposed (row=j, col=i); causal keeps j <= i => triu
    tri = np.triu(np.ones((128, 128), dtype=np.float32))
    in_maps = []
    for c in range(NCORES):
        b, g = divmod(c, TP)
        cs = slice(g * EG, (g + 1) * EG)
        in_maps.append(
            {
                "xT": np.ascontiguousarray(np.asarray(x)[b].T).astype(np.float32),
                "wq": np.ascontiguousarray(np.asarray(Wq)[:, cs]),
                "wk": np.ascontiguousarray(np.asarray(Wk)[:, cs]),
                "wv": np.ascontiguousarray(np.asarray(Wv)[:, cs]),
                "wo": np.ascontiguousarray(np.asarray(Wo)[cs, :]),
                "bq": np.ascontiguousarray(np.asarray(bq)[cs]),
                "bk": np.ascontiguousarray(np.asarray(bk)[cs]),
                "bv": np.ascontiguousarray(np.asarray(bv)[cs]),
                "trimask": tri,
            }
        )
    return in_maps


def gather(results, bo):
    bo = np.asarray(bo)
    outs = []
    for b in range(B):
        acc = np.zeros((S, E), dtype=np.float64)
        for g in range(TP):
            acc += results[b * TP + g]["out"].astype(np.float64)
        outs.append((acc + bo.astype(np.float64)).astype(np.float32))
    return np.stack(outs)


def run(inputs, trace=False, tmpdir=None):
    from concourse.bass_utils import run_bass_kernel_spmd

    nc = _get_nc()
    in_maps = make_in_maps(**inputs)
    res = run_bass_kernel_spmd(
        nc, in_maps, list(range(NCORES)), trace=trace, tmpdir=tmpdir
    )
    return gather(res.results, inputs["bo"]), res


def kernel(**inputs) -> np.ndarray:
    out, _ = run(inputs, trace=False)
    return out
